# revision 32
# baseline (speedup 1.0000x reference)
"""Trainium2 Bass kernel for a single AttnDecoderRNN step (batch=1).

Strategy: batch=1 leaves no useful sharding — the whole step runs on one
NeuronCore (core 0). Everything the device needs (weights, hidden state,
encoder outputs, biases) is re-laid-out on the host into a single [128, F]
bf16 "mega" array whose columns are exactly the SBUF images the tensor engine
wants (W^T 128-row chunks for the stationary operand, activations and biases
in column-major [128, c] layout; f32 biases are bit-packed into bf16 column
pairs and bitcast back on device). The device then:

  1. DMAs the mega array in 5 layer-ordered chunks so compute on early layers
     overlaps the DMA of later ones (one DMA ring, minimal issue overhead).
  2. Runs every matvec on the tensor engine in "column" dataflow:
     out[m] = sum_k W^T[k, m] * x[k]  with x as the moving operand, so
     activations stay in [128, c] column layout end-to-end (no transposes).
  3. Uses one activation table set (natural_log_exp_and_others): sigmoid and
     tanh are computed from exp + vector-engine reciprocal; softmax /
     log_softmax use exp/ln with matmul-against-ones partition reductions.
  4. Matmul path runs in bf16 (halves DMA bytes, single-pass PE); gate math,
     biases and all outputs stay fp32.

Embedding row, the two one-hot-selected w_l2d columns, and the is_head branch
are resolved at trace time on the host (they are index selections, not
arithmetic); all floating-point math on activations happens on the device.
"""

import numpy as np
import ml_dtypes

H = 256
L = 20
V = 29

_F32 = np.float32
_BF16 = ml_dtypes.bfloat16

# ---------------------------------------------------------------- mega layout
# Sections, in first-use order. Each entry: (name, n_cols of bf16).
_SECTIONS = [
    ("hid", 2),       # hidden state, column layout [128,2]
    ("biasp", 36),    # fp32 bias block bit-packed as bf16 pairs (18 f32 cols):
                      # f32 cols 0: attn_b | 1-2: comb_b | 3: out_b | 4-7: b_rz
                      # | 8-9: b_hn | 10-11: b_in | 12-13: w_l2dT[256+c0]
                      # | 14-15: w_l2dT[260+c1] | 16-17: b_l2d
    ("l2d_w", 512),   # w_l2d[:, :256].T chunks: block (kc, mc) at kc*256+mc*128
    ("emb", 2),       # emb[idx] in column layout [128,2]
    ("attn_w", 80),   # w_attn.T [512,20] chunks: block kc at kc*20
    ("ones", 130),    # all-ones block: col [*,1] + rows up to [1,128]
    ("enc", 256),     # encoder_outputs [20,256] in rows 0:20
    ("comb_w", 1024), # w_comb.T [512,256] chunks: block (kc,mc) at kc*256+mc*128
    ("ih_w", 1536),   # w_ih.T [256,768] chunks: block (kc,mc) at kc*768+mc*128
    ("hh_w", 1536),   # w_hh.T [256,768] chunks
    ("out_wf", 116),  # w_out.T [256,29] chunks, stored fp32 (2 bf16 cols per f32)
    ("idf", 256),     # fp32 128x128 identity bit-packed (row extraction)
]
_OFF = {}
_cursor = 0
for _name, _n in _SECTIONS:
    _OFF[_name] = _cursor
    _cursor += _n
F_TOTAL = _cursor  # 5072

# DMA group boundaries (column ranges of mega), in layer order.
_GROUPS = [
    (0, _OFF["attn_w"]),                 # hid + biases + l2d + emb   (552)
    (_OFF["attn_w"], _OFF["comb_w"]),    # attn + ones + enc          (366)
    (_OFF["comb_w"], _OFF["ih_w"]),      # comb                       (1024)
    (_OFF["ih_w"], F_TOTAL),             # ih + hh + out + identity   (3386)
]


def _col(v):
    """[128*c] vector -> [128, c] column-major image."""
    v = np.asarray(v, _F32)
    return np.ascontiguousarray(v.reshape(-1, 128).T)


def _colpad(v):
    """[n<=128] vector -> [128, 1], zero padded."""
    out = np.zeros((128, 1), _F32)
    v = np.asarray(v, _F32)
    out[: v.shape[0], 0] = v
    return out


def _chunks(wT):
    """wT [K, O] (K % 128 == 0) -> [128, (K//128)*O] stacked k-chunks."""
    K, O = wT.shape
    return np.concatenate(
        [wT[kc * 128 : (kc + 1) * 128, :] for kc in range(K // 128)], axis=1
    )


def _build_mega(input_tensor, hidden, encoder_outputs, cond, emb, w_l2d,
                b_l2d, w_attn, b_attn, w_comb, b_comb, w_ih, w_hh, b_ih,
                b_hh, w_out, b_out):
    idx = int(np.asarray(input_tensor).reshape(-1)[0])
    c0 = int(np.asarray(cond).reshape(-1)[0])
    c1 = int(np.asarray(cond).reshape(-1)[1])

    mega = np.zeros((128, F_TOTAL), _BF16)

    def put(name, arr):
        off = _OFF[name]
        mega[: arr.shape[0], off : off + arr.shape[1]] = arr.astype(_BF16)

    put("hid", _col(np.asarray(hidden, _F32).reshape(H)))

    w_l2dT = np.ascontiguousarray(np.asarray(w_l2d, _F32).T)  # [264, 256]
    b_ih = np.asarray(b_ih, _F32)
    b_hh = np.asarray(b_hh, _F32)
    biasf = np.zeros((128, 18), _F32)
    biasf[:, 0:1] = _colpad(b_attn)
    biasf[:, 1:3] = _col(b_comb)
    biasf[:, 3:4] = _colpad(b_out)
    biasf[:, 4:8] = _col((b_ih + b_hh)[:512])
    biasf[:, 8:10] = _col(b_hh[512:])
    biasf[:, 10:12] = _col(b_ih[512:])
    biasf[:, 12:14] = _col(w_l2dT[256 + c0])
    biasf[:, 14:16] = _col(w_l2dT[260 + c1])
    biasf[:, 16:18] = _col(b_l2d)
    # bit-preserving pack: each f32 column becomes two bf16 columns
    off = _OFF["biasp"]
    mega[:, off : off + 36] = np.ascontiguousarray(biasf).view(_BF16)

    put("l2d_w", _chunks(w_l2dT[:256, :]))
    put("emb", _col(np.asarray(emb, _F32)[idx]))
    put("attn_w", _chunks(np.ascontiguousarray(np.asarray(w_attn, _F32).T)))
    put("ones", np.ones((128, 130), _F32))
    put("enc", np.asarray(encoder_outputs, _F32))
    put("comb_w", _chunks(np.ascontiguousarray(np.asarray(w_comb, _F32).T)))
    put("ih_w", _chunks(np.ascontiguousarray(np.asarray(w_ih, _F32).T)))
    put("hh_w", _chunks(np.ascontiguousarray(np.asarray(w_hh, _F32).T)))
    off = _OFF["out_wf"]
    mega[:, off : off + 116] = np.ascontiguousarray(
        _chunks(np.ascontiguousarray(np.asarray(w_out, _F32).T))).view(_BF16)
    off = _OFF["idf"]
    mega[:, off : off + 256] = np.eye(128, dtype=_F32).view(_BF16)
    # DMA cost scales with element count, not bytes: ship as f32 (same bits,
    # half the elements -> ~2x stream rate), bitcast back to bf16 on device.
    return np.ascontiguousarray(mega).view(_F32)


# ---------------------------------------------------------------- bass program
_CACHE = {}


def _split_multiwaits(nc):
    """The pinned walrus build rejects >1 sync wait per instruction.  Split
    every multi-wait instruction into a chain of single-wait NOPs on the same
    engine (sequential waits on one engine == logical AND)."""
    import concourse.mybir as mybir

    def fix_block(b):
        insts = b.instructions
        i = 0
        while i < len(insts):
            inst = insts[i]
            si = getattr(inst, "sync_info", None)
            waits = list(si.on_wait) if si is not None and si.on_wait else []
            if len(waits) > 1:
                inst.sync_info = mybir.SyncInfo(
                    on_update=list(si.on_update or []), on_wait=waits[-1:]
                )
                nops = [
                    mybir.InstNoOp(
                        name=f"{inst.name}-sw{k}",
                        engine=inst.engine,
                        bass_nofuse=True,
                        sync_info=mybir.SyncInfo(on_update=[], on_wait=[w]),
                    )
                    for k, w in enumerate(waits[:-1])
                ]
                insts[i:i] = nops
                i += len(nops)
            i += 1
        for sub in getattr(b, "blocks", []) or []:
            fix_block(sub)

    for fn in nc.m.functions:
        for b in fn.blocks:
            fix_block(b)


def _hoist_dmas(nc):
    """Move the mega weight-load DMAs from the Tile body block into the
    preamble ("main") block, ahead of the engine-entry barriers: the SP
    engine then issues them ~0.7us earlier and the transfers overlap the
    rest of the preamble.  Safe because DMA completion is carried by
    semaphore increments that nothing in the preamble resets."""
    fn = nc.m.functions[0]
    pre, body = fn.blocks[0], fn.blocks[1]
    dmas = [i for i in body.instructions
            if i.__class__.__name__ == "InstDMACopy"
            and getattr(i.ins[0], "memref", None) == "mega"][:4]
    if not dmas:
        return
    binsts = body.instructions
    for d in dmas:
        binsts.remove(d)
    pinsts = pre.instructions
    pos = next(
        (k for k, i in enumerate(pinsts)
         if i.__class__.__name__ == "InstEventSemaphore"),
        len(pinsts),
    )
    pinsts[pos:pos] = dmas


def _build_program(is_head, zb):
    import concourse.bass as bass
    import concourse.mybir as mybir
    import concourse.tile as tile

    f32 = mybir.dt.float32
    wdt = mybir.dt.bfloat16
    AF = mybir.ActivationFunctionType
    ALU = mybir.AluOpType

    assert F_TOTAL % 2 == 0
    nc = bass.Bass()
    mega_d = nc.dram_tensor("mega", [128, F_TOTAL // 2], f32,
                            kind="ExternalInput")
    # single packed output: row0 = [logits pad32 | attn pad32 | h[0:128]],
    # row1 = [64 pad | h[128:256]]
    out_d = nc.dram_tensor("out", [2, 192], f32, kind="ExternalOutput")

    with tile.TileContext(nc) as tc:
        with (
            nc.allow_low_precision(reason="bf16 matmul path"),
            tc.tile_pool(name="w", bufs=1) as wp,
            tc.tile_pool(name="a", bufs=1) as ap,
            tc.tile_pool(name="ps", bufs=1, space="PSUM") as pp,
        ):
            gtiles = []
            gviews = []
            for gi_, (lo, hi) in enumerate(_GROUPS):
                t = wp.tile([128, (hi - lo) // 2], f32, tag=f"g{gi_}")
                nc.sync.dma_start(t[:], mega_d[:, lo // 2 : hi // 2])
                gtiles.append(t)
                gviews.append(None)

            def sec(name, r0, r1, p0=0, p1=128):
                """bf16 view of a mega section slice (cols in bf16 units)."""
                off = _OFF[name]
                for g, (lo, hi) in zip(gtiles, _GROUPS):
                    if lo <= off < hi:
                        return g.bitcast(wdt)[p0:p1,
                                              off - lo + r0 : off - lo + r1]
                raise KeyError(name)

            def f32sec(name, r0, r1, p0=0, p1=128):
                """native f32 slice (cols in f32 units)."""
                off = _OFF[name]
                for g, (lo, hi) in zip(gtiles, _GROUPS):
                    if lo <= off < hi:
                        o = (off - lo) // 2
                        return g[p0:p1, o + r0 : o + r1]
                raise KeyError(name)

            def bia(j0, j1, p0=0, p1=128):
                return f32sec("biasp", j0, j1, p0, p1)

            big = ap.tile([32, 192], f32)  # packed output rows
            nc.vector.memset(big[:], 0.0)
            # ---- stage 0: h' = w_l2d[:, :256] @ h + (w_l2d col c0 + col c1 + b_l2d)
            hp = ap.tile([128, 2], wdt)  # h' column layout
            hid_c = sec("hid", 0, 2)
            if is_head:
                ebias = ap.tile([128, 2], f32)
                eb1 = ap.tile([128, 2], f32)
                nc.vector.tensor_add(eb1[:], bia(12, 14), bia(14, 16))
                nc.vector.tensor_add(ebias[:], eb1[:], bia(16, 18))
                ps_h = pp.tile([128, 2], f32)
                for mc in range(2):
                    for kc in range(2):
                        nc.tensor.matmul(
                            ps_h[:, mc : mc + 1],
                            sec("l2d_w", kc * 256 + mc * 128,
                                kc * 256 + mc * 128 + 128),
                            hid_c[:, kc : kc + 1],
                            start=(kc == 0), stop=(kc == 1),
                        )
                nc.vector.tensor_add(hp[:], ps_h[:], ebias[:])
            else:
                nc.vector.tensor_copy(hp[:], hid_c)

            # ---- stage 1: attention weights = softmax(w_attn @ [e; h'] + b_attn)
            ps_sm = pp.tile([L, 24], f32)  # 0: logits, 1: sum, 2: bcast, 4:24 row
            e0 = sec("emb", 0, 1)
            e1 = sec("emb", 1, 2)
            rhs_attn = [e0, e1, hp[:, 0:1], hp[:, 1:2]]
            for kc in range(4):
                nc.tensor.matmul(
                    ps_sm[0:L, 0:1],
                    sec("attn_w", kc * 20, kc * 20 + 20),
                    rhs_attn[kc],
                    start=(kc == 0), stop=(kc == 3),
                )
            exp_a = ap.tile([L, 1], wdt)
            nc.scalar.activation(exp_a[:], ps_sm[0:L, 0:1], AF.Exp,
                                 bias=(0.0 if zb[0] else bia(0, 1, 0, L)))
            # attn_applied computed on UNNORMALIZED exp weights; the 1/sum
            # factor is applied to the result via a per-partition scalar.
            ps_ap = pp.tile([128, 132], f32)  # 0-1: ap_raw, 2: 1/sum bcast, 4:132 h row
            for mc in range(2):
                nc.tensor.matmul(
                    ps_ap[:, mc : mc + 1],
                    sec("enc", mc * 128, (mc + 1) * 128, 0, L),
                    exp_a[0:L, 0:1],
                    start=True, stop=True,
                )
            nc.tensor.matmul(ps_sm[0:1, 1:2], exp_a[:], sec("ones", 0, 1, 0, L),
                             start=True, stop=True)
            rsum = ap.tile([1, 1], wdt)
            nc.vector.reciprocal(rsum[:], ps_sm[0:1, 1:2])
            nc.tensor.matmul(ps_ap[0:128, 2:3], sec("ones", 2, 130, 0, 1),
                             rsum[:], start=True, stop=True)
            bc_sb = ap.tile([128, 1], f32)
            nc.vector.tensor_copy(bc_sb[:], ps_ap[0:128, 2:3])
            ap_sb = ap.tile([128, 2], wdt)
            nc.vector.tensor_scalar_mul(ap_sb[:], ps_ap[:, 0:2], bc_sb[:])
            # attention-weights output row (off critical path)
            attn_f = ap.tile([32, 32], f32)
            nc.vector.memset(attn_f[:], 0.0)
            nc.vector.tensor_scalar_mul(attn_f[0:L, 0:1], exp_a[:],
                                        bc_sb[0:L, 0:1])
            nc.vector.transpose(big[0:32, 32:64], attn_f[:])

            # ---- stage 3: u = relu(w_comb @ [e; attn_applied] + b_comb)
            ps_u = pp.tile([128, 2], f32)
            rhs_comb = [e0, e1, ap_sb[:, 0:1], ap_sb[:, 1:2]]
            for mc in range(2):
                for kc in range(4):
                    nc.tensor.matmul(
                        ps_u[:, mc : mc + 1],
                        sec("comb_w", kc * 256 + mc * 128,
                            kc * 256 + mc * 128 + 128),
                        rhs_comb[kc],
                        start=(kc == 0), stop=(kc == 3),
                    )
            u = ap.tile([128, 2], wdt)
            if zb[1]:
                nc.scalar.activation(u[:], ps_u[:, 0:2], AF.Relu)
            else:
                tu = ap.tile([128, 2], f32)
                nc.vector.tensor_add(tu[:], ps_u[:, 0:2], bia(1, 3))
                nc.scalar.activation(u[:], tu[:], AF.Relu)

            # ---- stage 4: GRU cell
            # r/z gates need gi+gh summed — accumulate both weight matmuls
            # into one PSUM group. The n-gate halves stay separate.
            ps_rz = pp.tile([128, 4], f32)
            ps_gin = pp.tile([128, 2], f32)
            ps_ghn = pp.tile([128, 2], f32)
            for mc in range(4):
                for kc in range(2):
                    nc.tensor.matmul(
                        ps_rz[:, mc : mc + 1],
                        sec("ih_w", kc * 768 + mc * 128,
                            kc * 768 + mc * 128 + 128),
                        u[:, kc : kc + 1],
                        start=(kc == 0), stop=False,
                    )
                for kc in range(2):
                    nc.tensor.matmul(
                        ps_rz[:, mc : mc + 1],
                        sec("hh_w", kc * 768 + mc * 128,
                            kc * 768 + mc * 128 + 128),
                        hp[:, kc : kc + 1],
                        start=False, stop=(kc == 1),
                    )
            for mc in range(4, 6):
                for kc in range(2):
                    nc.tensor.matmul(
                        ps_gin[:, mc - 4 : mc - 3],
                        sec("ih_w", kc * 768 + mc * 128,
                            kc * 768 + mc * 128 + 128),
                        u[:, kc : kc + 1],
                        start=(kc == 0), stop=(kc == 1),
                    )
                for kc in range(2):
                    nc.tensor.matmul(
                        ps_ghn[:, mc - 4 : mc - 3],
                        sec("hh_w", kc * 768 + mc * 128,
                            kc * 768 + mc * 128 + 128),
                        hp[:, kc : kc + 1],
                        start=(kc == 0), stop=(kc == 1),
                    )
            # r, z = sigmoid(gi[0:512] + gh[0:512] + b_rz) via exp + reciprocal
            e_rz = ap.tile([128, 4], f32)
            if zb[2]:
                nc.scalar.activation(e_rz[:], ps_rz[:], AF.Exp, scale=-1.0)
            else:
                t2 = ap.tile([128, 4], f32)
                nc.vector.tensor_add(t2[:], ps_rz[:], bia(4, 8))
                nc.scalar.activation(e_rz[:], t2[:], AF.Exp, scale=-1.0)
            p_rz = ap.tile([128, 4], f32)
            nc.vector.tensor_scalar_add(p_rz[:], e_rz[:], 1.0)
            rz = ap.tile([128, 4], f32)
            nc.vector.reciprocal(rz[:], p_rz[:])
            # n = tanh(gi[512:768] + b_in + r * (gh[512:768] + b_hn))
            rhn = ap.tile([128, 2], f32)
            if zb[3]:
                nc.vector.tensor_mul(rhn[:], rz[:, 0:2], ps_ghn[:])
            else:
                hn = ap.tile([128, 2], f32)
                nc.vector.tensor_add(hn[:], ps_ghn[:], bia(8, 10))
                nc.vector.tensor_mul(rhn[:], rz[:, 0:2], hn[:])
            vn = ap.tile([128, 2], f32)
            if zb[4]:
                nc.vector.tensor_add(vn[:], ps_gin[:], rhn[:])
            else:
                t3 = ap.tile([128, 2], f32)
                nc.vector.tensor_add(t3[:], ps_gin[:], bia(10, 12))
                nc.vector.tensor_add(vn[:], t3[:], rhn[:])
            e_n = ap.tile([128, 2], f32)
            nc.scalar.activation(e_n[:], vn[:], AF.Exp, scale=-2.0)
            p_n = ap.tile([128, 2], f32)
            nc.vector.tensor_scalar_add(p_n[:], e_n[:], 1.0)
            r_n = ap.tile([128, 2], f32)
            nc.vector.reciprocal(r_n[:], p_n[:])
            n_t = ap.tile([128, 2], f32)   # n = 2*r - 1
            nc.vector.tensor_scalar(n_t[:], r_n[:], 2.0, 1.0,
                                    ALU.mult, ALU.subtract)
            # h_new = n + z * (h' - n)
            d_t = ap.tile([128, 2], f32)
            nc.vector.tensor_sub(d_t[:], hp[:], n_t[:])
            zd = ap.tile([128, 2], f32)
            nc.vector.tensor_mul(zd[:], rz[:, 2:4], d_t[:])
            h_new = ap.tile([128, 2], f32)
            nc.vector.tensor_add(h_new[:], n_t[:], zd[:])
            # ---- stage 5: logits = log_softmax(w_out @ h_new + b_out)
            # out_w kept fp32 so the matmul reads h_new directly (no cast)
            ps_l = pp.tile([V, 1], f32)
            for kc in range(2):
                nc.tensor.matmul(
                    ps_l[0:V, 0:1],
                    f32sec("out_wf", kc * 29, kc * 29 + 29),
                    h_new[:, kc : kc + 1],
                    start=(kc == 0), stop=(kc == 1),
                )
            # h_new row extraction (bf16 matmul against identity), packed into
            # the output tile; emitted after the logits matmuls.
            nc.tensor.matmul(ps_ap[0:2, 4:132], h_new[0:128, 0:2],
                             f32sec("idf", 0, 128), start=True, stop=True)
            nc.vector.tensor_copy(big[0:2, 64:192], ps_ap[0:2, 4:132])
            # log-softmax in row form: transpose first, sum via ACT accum_out
            l_pad = ap.tile([32, 32], f32)
            nc.vector.memset(l_pad[:], 0.0)
            if zb[5]:
                nc.vector.tensor_copy(l_pad[0:V, 0:1], ps_l[0:V, 0:1])
            else:
                nc.vector.tensor_add(l_pad[0:V, 0:1], ps_l[0:V, 0:1],
                                     bia(3, 4, 0, V))
            l_row = ap.tile([32, 32], f32)
            nc.vector.transpose(l_row[:], l_pad[:])
            e_row = ap.tile([1, 32], f32)
            sx = ap.tile([1, 1], f32)
            nc.scalar.activation(e_row[0:1, 0:V], l_row[0:1, 0:V], AF.Exp,
                                 accum_out=sx[:])
            lse = ap.tile([1, 1], f32)
            nc.scalar.activation(lse[:], sx[:], AF.Ln)
            nc.vector.tensor_scalar(big[0:1, 0:V], l_row[0:1, 0:V], lse[0:1, 0:1],
                                    None, ALU.subtract)
            nc.sync.dma_start(out_d[0:2, 0:192], big[0:2, 0:192])

    _split_multiwaits(nc)
    _hoist_dmas(nc)
    return nc


# ---------------------------------------------------------------- entry point
LAST_EXEC_NS = None


def kernel(input_tensor, hidden, encoder_outputs, cond, is_head,
           emb, w_l2d, b_l2d, w_attn, b_attn, w_comb, b_comb,
           w_ih, w_hh, b_ih, b_hh, w_out, b_out, _trace=False):
    global LAST_EXEC_NS
    from concourse.bass_utils import run_bass_kernel_spmd

    head = bool(np.asarray(is_head).reshape(-1)[0] if np.asarray(is_head).size
                else is_head)
    zb = (
        not np.any(np.asarray(b_attn)),          # attn bias zero
        not np.any(np.asarray(b_comb)),          # comb bias zero
        not (np.any(np.asarray(b_ih)[:512]) or np.any(np.asarray(b_hh)[:512])),
        not np.any(np.asarray(b_hh)[512:]),      # b_hn zero
        not np.any(np.asarray(b_ih)[512:]),      # b_in zero
        not np.any(np.asarray(b_out)),           # out bias zero
    )
    key = ("p", head, zb)
    if key not in _CACHE:
        _CACHE[key] = _build_program(head, zb)
    nc = _CACHE[key]

    mega = _build_mega(input_tensor, hidden, encoder_outputs, cond, emb,
                       w_l2d, b_l2d, w_attn, b_attn, w_comb, b_comb,
                       w_ih, w_hh, b_ih, b_hh, w_out, b_out)
    res = run_bass_kernel_spmd(nc, [{"mega": mega}], core_ids=[0],
                               trace=_trace)
    LAST_EXEC_NS = res.exec_time_ns
    out = np.asarray(res.results[0]["out"], _F32)
    logits = out[0, 0:V].reshape(1, V).copy()
    attn_out = out[0, 32 : 32 + L].reshape(1, L).copy()
    h_out = np.concatenate([out[0, 64:192], out[1, 64:192]]).reshape(1, 1, H)
    return logits, h_out, attn_out


# revision 33
# speedup vs baseline: 1.1273x; 1.1273x over previous
"""Trainium2 Bass kernel for a single AttnDecoderRNN step (batch=1).

Strategy: batch=1 leaves no useful sharding — the whole step runs on one
NeuronCore (core 0). Everything the device needs (weights, hidden state,
encoder outputs, biases) is re-laid-out on the host into a single [128, F]
bf16 "mega" array whose columns are exactly the SBUF images the tensor engine
wants (W^T 128-row chunks for the stationary operand, activations and biases
in column-major [128, c] layout; f32 biases are bit-packed into bf16 column
pairs and bitcast back on device). The device then:

  1. DMAs the mega array in 5 layer-ordered chunks so compute on early layers
     overlaps the DMA of later ones (one DMA ring, minimal issue overhead).
  2. Runs every matvec on the tensor engine in "column" dataflow:
     out[m] = sum_k W^T[k, m] * x[k]  with x as the moving operand, so
     activations stay in [128, c] column layout end-to-end (no transposes).
  3. Uses one activation table set (natural_log_exp_and_others): sigmoid and
     tanh are computed from exp + vector-engine reciprocal; softmax /
     log_softmax use exp/ln with matmul-against-ones partition reductions.
  4. Matmul path runs in bf16 (halves DMA bytes, single-pass PE); gate math,
     biases and all outputs stay fp32.

Embedding row, the two one-hot-selected w_l2d columns, and the is_head branch
are resolved at trace time on the host (they are index selections, not
arithmetic); all floating-point math on activations happens on the device.
"""

import numpy as np
import ml_dtypes

H = 256
L = 20
V = 29

_F32 = np.float32
_BF16 = ml_dtypes.bfloat16

# ---------------------------------------------------------------- mega layout
# Sections, in first-use order. Each entry: (name, n_cols of bf16).
_SECTIONS = [
    ("hid", 2),       # hidden state, column layout [128,2]
    ("biasp", 36),    # fp32 bias block bit-packed as bf16 pairs (18 f32 cols):
                      # f32 cols 0: attn_b | 1-2: comb_b | 3: out_b | 4-7: b_rz
                      # | 8-9: b_hn | 10-11: b_in | 12-13: w_l2dT[256+c0]
                      # | 14-15: w_l2dT[260+c1] | 16-17: b_l2d
    ("l2d_w", 512),   # w_l2d[:, :256].T chunks: block (kc, mc) at kc*256+mc*128
    ("emb", 2),       # emb[idx] in column layout [128,2]
    ("attn_w", 80),   # w_attn.T [512,20] chunks: block kc at kc*20
    ("ones", 130),    # all-ones block: col [*,1] + rows up to [1,128]
    ("enc", 256),     # encoder_outputs [20,256] in rows 0:20
    ("comb_w", 1024), # w_comb.T [512,256] chunks: block (kc,mc) at kc*256+mc*128
    ("ih_w", 1536),   # w_ih.T [256,768] chunks: block (kc,mc) at kc*768+mc*128
    ("hh_w", 1536),   # w_hh.T [256,768] chunks
    ("out_wf", 116),  # w_out.T [256,29] chunks, stored fp32 (2 bf16 cols per f32)
    ("idf", 256),     # fp32 128x128 identity bit-packed (row extraction)
]
_OFF = {}
_cursor = 0
for _name, _n in _SECTIONS:
    _OFF[_name] = _cursor
    _cursor += _n
F_TOTAL = _cursor  # 5072

# DMA group boundaries (column ranges of mega), in layer order.
_GROUPS = [
    (0, _OFF["attn_w"]),                 # hid + biases + l2d + emb   (552)
    (_OFF["attn_w"], _OFF["comb_w"]),    # attn + ones + enc          (366)
    (_OFF["comb_w"], _OFF["ih_w"]),      # comb                       (1024)
    (_OFF["ih_w"], F_TOTAL),             # ih + hh + out + identity   (3386)
]


def _col(v):
    """[128*c] vector -> [128, c] column-major image."""
    v = np.asarray(v, _F32)
    return np.ascontiguousarray(v.reshape(-1, 128).T)


def _colpad(v):
    """[n<=128] vector -> [128, 1], zero padded."""
    out = np.zeros((128, 1), _F32)
    v = np.asarray(v, _F32)
    out[: v.shape[0], 0] = v
    return out


def _chunks(wT):
    """wT [K, O] (K % 128 == 0) -> [128, (K//128)*O] stacked k-chunks."""
    K, O = wT.shape
    return np.concatenate(
        [wT[kc * 128 : (kc + 1) * 128, :] for kc in range(K // 128)], axis=1
    )


def _build_mega(input_tensor, hidden, encoder_outputs, cond, emb, w_l2d,
                b_l2d, w_attn, b_attn, w_comb, b_comb, w_ih, w_hh, b_ih,
                b_hh, w_out, b_out):
    idx = int(np.asarray(input_tensor).reshape(-1)[0])
    c0 = int(np.asarray(cond).reshape(-1)[0])
    c1 = int(np.asarray(cond).reshape(-1)[1])

    mega = np.zeros((128, F_TOTAL), _BF16)

    def put(name, arr):
        off = _OFF[name]
        mega[: arr.shape[0], off : off + arr.shape[1]] = arr.astype(_BF16)

    put("hid", _col(np.asarray(hidden, _F32).reshape(H)))

    w_l2dT = np.ascontiguousarray(np.asarray(w_l2d, _F32).T)  # [264, 256]
    b_ih = np.asarray(b_ih, _F32)
    b_hh = np.asarray(b_hh, _F32)
    biasf = np.zeros((128, 18), _F32)
    biasf[:, 0:1] = _colpad(b_attn)
    biasf[:, 1:3] = _col(b_comb)
    biasf[:, 3:4] = _colpad(b_out)
    biasf[:, 4:8] = _col((b_ih + b_hh)[:512])
    biasf[:, 8:10] = _col(b_hh[512:])
    biasf[:, 10:12] = _col(b_ih[512:])
    biasf[:, 12:14] = _col(w_l2dT[256 + c0])
    biasf[:, 14:16] = _col(w_l2dT[260 + c1])
    biasf[:, 16:18] = _col(b_l2d)
    # bit-preserving pack: each f32 column becomes two bf16 columns
    off = _OFF["biasp"]
    mega[:, off : off + 36] = np.ascontiguousarray(biasf).view(_BF16)

    put("l2d_w", _chunks(w_l2dT[:256, :]))
    put("emb", _col(np.asarray(emb, _F32)[idx]))
    put("attn_w", _chunks(np.ascontiguousarray(np.asarray(w_attn, _F32).T)))
    put("ones", np.ones((128, 130), _F32))
    put("enc", np.asarray(encoder_outputs, _F32))
    put("comb_w", _chunks(np.ascontiguousarray(np.asarray(w_comb, _F32).T)))
    put("ih_w", _chunks(np.ascontiguousarray(np.asarray(w_ih, _F32).T)))
    put("hh_w", _chunks(np.ascontiguousarray(np.asarray(w_hh, _F32).T)))
    off = _OFF["out_wf"]
    mega[:, off : off + 116] = np.ascontiguousarray(
        _chunks(np.ascontiguousarray(np.asarray(w_out, _F32).T))).view(_BF16)
    off = _OFF["idf"]
    mega[:, off : off + 256] = np.eye(128, dtype=_F32).view(_BF16)
    # DMA cost scales with element count, not bytes: ship as f32 (same bits,
    # half the elements -> ~2x stream rate), bitcast back to bf16 on device.
    return np.ascontiguousarray(mega).view(_F32)


# ---------------------------------------------------------------- bass program
_CACHE = {}


def _split_multiwaits(nc):
    """The pinned walrus build rejects >1 sync wait per instruction.  Split
    every multi-wait instruction into a chain of single-wait NOPs on the same
    engine (sequential waits on one engine == logical AND)."""
    import concourse.mybir as mybir

    def fix_block(b):
        insts = b.instructions
        i = 0
        while i < len(insts):
            inst = insts[i]
            si = getattr(inst, "sync_info", None)
            waits = list(si.on_wait) if si is not None and si.on_wait else []
            if len(waits) > 1:
                inst.sync_info = mybir.SyncInfo(
                    on_update=list(si.on_update or []), on_wait=waits[-1:]
                )
                nops = [
                    mybir.InstNoOp(
                        name=f"{inst.name}-sw{k}",
                        engine=inst.engine,
                        bass_nofuse=True,
                        sync_info=mybir.SyncInfo(on_update=[], on_wait=[w]),
                    )
                    for k, w in enumerate(waits[:-1])
                ]
                insts[i:i] = nops
                i += len(nops)
            i += 1
        for sub in getattr(b, "blocks", []) or []:
            fix_block(sub)

    for fn in nc.m.functions:
        for b in fn.blocks:
            fix_block(b)


def _hoist_dmas(nc):
    """Move the mega weight-load DMAs from the Tile body block into the
    preamble ("main") block, ahead of the engine-entry barriers: the SP
    engine then issues them ~0.7us earlier and the transfers overlap the
    rest of the preamble.  Safe because DMA completion is carried by
    semaphore increments that nothing in the preamble resets."""
    fn = nc.m.functions[0]
    pre, body = fn.blocks[0], fn.blocks[1]
    dmas = [i for i in body.instructions
            if i.__class__.__name__ == "InstDMACopy"
            and getattr(i.ins[0], "memref", None) == "mega"][:4]
    if not dmas:
        return
    binsts = body.instructions
    for d in dmas:
        binsts.remove(d)
    pinsts = pre.instructions
    pos = next(
        (k for k, i in enumerate(pinsts)
         if i.__class__.__name__ == "InstEventSemaphore"),
        len(pinsts),
    )
    pinsts[pos:pos] = dmas


def _build_program(is_head, zb):
    import concourse.bass as bass
    import concourse.mybir as mybir
    import concourse.tile as tile

    f32 = mybir.dt.float32
    wdt = mybir.dt.bfloat16
    AF = mybir.ActivationFunctionType
    ALU = mybir.AluOpType

    assert F_TOTAL % 2 == 0
    nc = bass.Bass()
    mega_d = nc.dram_tensor("mega", [128, F_TOTAL // 2], f32,
                            kind="ExternalInput")
    # single packed output: row0 = [logits pad32 | attn pad32 | h[0:128]],
    # row1 = [64 pad | h[128:256]]
    out_d = nc.dram_tensor("out", [2, 192], f32, kind="ExternalOutput")

    with tile.TileContext(nc) as tc:
        with (
            nc.allow_low_precision(reason="bf16 matmul path"),
            tc.tile_pool(name="w", bufs=1) as wp,
            tc.tile_pool(name="a", bufs=1) as ap,
            tc.tile_pool(name="ps", bufs=1, space="PSUM") as pp,
        ):
            gtiles = []
            gviews = []
            for gi_, (lo, hi) in enumerate(_GROUPS):
                t = wp.tile([128, (hi - lo) // 2], f32, tag=f"g{gi_}")
                nc.sync.dma_start(t[:], mega_d[:, lo // 2 : hi // 2])
                gtiles.append(t)
                gviews.append(None)

            def sec(name, r0, r1, p0=0, p1=128):
                """bf16 view of a mega section slice (cols in bf16 units)."""
                off = _OFF[name]
                for g, (lo, hi) in zip(gtiles, _GROUPS):
                    if lo <= off < hi:
                        return g.bitcast(wdt)[p0:p1,
                                              off - lo + r0 : off - lo + r1]
                raise KeyError(name)

            def f32sec(name, r0, r1, p0=0, p1=128):
                """native f32 slice (cols in f32 units)."""
                off = _OFF[name]
                for g, (lo, hi) in zip(gtiles, _GROUPS):
                    if lo <= off < hi:
                        o = (off - lo) // 2
                        return g[p0:p1, o + r0 : o + r1]
                raise KeyError(name)

            def bia(j0, j1, p0=0, p1=128):
                return f32sec("biasp", j0, j1, p0, p1)

            # Dummy early exp so the ACT table set (natural_log_exp) loads
            # while the weight DMAs stream.
            dummy = ap.tile([1, 2], f32)
            nc.vector.memset(dummy[:], 0.0)
            nc.scalar.activation(dummy[:, 0:1], dummy[:, 0:1], AF.Exp)

            big = ap.tile([32, 192], f32)  # packed output rows
            nc.vector.memset(big[:], 0.0)
            # ---- stage 0: h' = w_l2d[:, :256] @ h + (w_l2d col c0 + col c1 + b_l2d)
            hp = ap.tile([128, 2], wdt)  # h' column layout
            hid_c = sec("hid", 0, 2)
            if is_head:
                ebias = ap.tile([128, 2], f32)
                eb1 = ap.tile([128, 2], f32)
                nc.vector.tensor_add(eb1[:], bia(12, 14), bia(14, 16))
                nc.vector.tensor_add(ebias[:], eb1[:], bia(16, 18))
                ps_h = pp.tile([128, 2], f32)
                for mc in range(2):
                    for kc in range(2):
                        nc.tensor.matmul(
                            ps_h[:, mc : mc + 1],
                            sec("l2d_w", kc * 256 + mc * 128,
                                kc * 256 + mc * 128 + 128),
                            hid_c[:, kc : kc + 1],
                            start=(kc == 0), stop=(kc == 1),
                        )
                nc.vector.tensor_add(hp[:], ps_h[:], ebias[:])
            else:
                nc.vector.tensor_copy(hp[:], hid_c)

            # ---- stage 1: attention weights = softmax(w_attn @ [e; h'] + b_attn)
            ps_sm = pp.tile([L, 24], f32)  # 0: logits, 1: sum, 2: bcast, 4:24 row
            e0 = sec("emb", 0, 1)
            e1 = sec("emb", 1, 2)
            rhs_attn = [e0, e1, hp[:, 0:1], hp[:, 1:2]]
            for kc in range(4):
                nc.tensor.matmul(
                    ps_sm[0:L, 0:1],
                    sec("attn_w", kc * 20, kc * 20 + 20),
                    rhs_attn[kc],
                    start=(kc == 0), stop=(kc == 3),
                )
            exp_a = ap.tile([L, 1], wdt)
            nc.scalar.activation(exp_a[:], ps_sm[0:L, 0:1], AF.Exp,
                                 bias=(0.0 if zb[0] else bia(0, 1, 0, L)))
            # attn_applied computed on UNNORMALIZED exp weights; the 1/sum
            # factor is applied to the result via a per-partition scalar.
            ps_ap = pp.tile([128, 132], f32)  # 0-1: ap_raw, 2: 1/sum bcast, 4:132 h row
            for mc in range(2):
                nc.tensor.matmul(
                    ps_ap[:, mc : mc + 1],
                    sec("enc", mc * 128, (mc + 1) * 128, 0, L),
                    exp_a[0:L, 0:1],
                    start=True, stop=True,
                )
            nc.tensor.matmul(ps_sm[0:1, 1:2], exp_a[:], sec("ones", 0, 1, 0, L),
                             start=True, stop=True)
            rsum = ap.tile([1, 1], wdt)
            nc.vector.reciprocal(rsum[:], ps_sm[0:1, 1:2])
            nc.tensor.matmul(ps_ap[0:128, 2:3], sec("ones", 2, 130, 0, 1),
                             rsum[:], start=True, stop=True)
            bc_sb = ap.tile([128, 1], f32)
            nc.vector.tensor_copy(bc_sb[:], ps_ap[0:128, 2:3])
            ap_sb = ap.tile([128, 2], wdt)
            nc.vector.tensor_scalar_mul(ap_sb[:], ps_ap[:, 0:2], bc_sb[:])
            # attention-weights output row (off critical path)
            attn_f = ap.tile([32, 32], f32)
            nc.vector.memset(attn_f[:], 0.0)
            nc.vector.tensor_scalar_mul(attn_f[0:L, 0:1], exp_a[:],
                                        bc_sb[0:L, 0:1])
            nc.vector.transpose(big[0:32, 32:64], attn_f[:])

            # ---- stage 3: u = relu(w_comb @ [e; attn_applied] + b_comb)
            ps_u = pp.tile([128, 2], f32)
            rhs_comb = [e0, e1, ap_sb[:, 0:1], ap_sb[:, 1:2]]
            for mc in range(2):
                for kc in range(4):
                    nc.tensor.matmul(
                        ps_u[:, mc : mc + 1],
                        sec("comb_w", kc * 256 + mc * 128,
                            kc * 256 + mc * 128 + 128),
                        rhs_comb[kc],
                        start=(kc == 0), stop=(kc == 3),
                    )
            u = ap.tile([128, 2], wdt)
            if zb[1]:
                nc.scalar.activation(u[:], ps_u[:, 0:2], AF.Relu)
            else:
                tu = ap.tile([128, 2], f32)
                nc.vector.tensor_add(tu[:], ps_u[:, 0:2], bia(1, 3))
                nc.scalar.activation(u[:], tu[:], AF.Relu)

            # ---- stage 4: GRU cell
            # r/z gates need gi+gh summed — accumulate both weight matmuls
            # into one PSUM group. The n-gate halves stay separate.
            ps_rz = pp.tile([128, 4], f32)
            ps_gin = pp.tile([128, 2], f32)
            ps_ghn = pp.tile([128, 2], f32)
            for mc in range(4):
                for kc in range(2):
                    nc.tensor.matmul(
                        ps_rz[:, mc : mc + 1],
                        sec("ih_w", kc * 768 + mc * 128,
                            kc * 768 + mc * 128 + 128),
                        u[:, kc : kc + 1],
                        start=(kc == 0), stop=False,
                    )
                for kc in range(2):
                    nc.tensor.matmul(
                        ps_rz[:, mc : mc + 1],
                        sec("hh_w", kc * 768 + mc * 128,
                            kc * 768 + mc * 128 + 128),
                        hp[:, kc : kc + 1],
                        start=False, stop=(kc == 1),
                    )
            for mc in range(4, 6):
                for kc in range(2):
                    nc.tensor.matmul(
                        ps_gin[:, mc - 4 : mc - 3],
                        sec("ih_w", kc * 768 + mc * 128,
                            kc * 768 + mc * 128 + 128),
                        u[:, kc : kc + 1],
                        start=(kc == 0), stop=(kc == 1),
                    )
                for kc in range(2):
                    nc.tensor.matmul(
                        ps_ghn[:, mc - 4 : mc - 3],
                        sec("hh_w", kc * 768 + mc * 128,
                            kc * 768 + mc * 128 + 128),
                        hp[:, kc : kc + 1],
                        start=(kc == 0), stop=(kc == 1),
                    )
            # r, z = sigmoid(gi[0:512] + gh[0:512] + b_rz) via exp + reciprocal
            e_rz = ap.tile([128, 4], f32)
            if zb[2]:
                nc.scalar.activation(e_rz[:], ps_rz[:], AF.Exp, scale=-1.0)
            else:
                t2 = ap.tile([128, 4], f32)
                nc.vector.tensor_add(t2[:], ps_rz[:], bia(4, 8))
                nc.scalar.activation(e_rz[:], t2[:], AF.Exp, scale=-1.0)
            p_rz = ap.tile([128, 4], f32)
            nc.vector.tensor_scalar_add(p_rz[:], e_rz[:], 1.0)
            rz = ap.tile([128, 4], f32)
            nc.vector.reciprocal(rz[:], p_rz[:])
            # n = tanh(gi[512:768] + b_in + r * (gh[512:768] + b_hn))
            rhn = ap.tile([128, 2], f32)
            if zb[3]:
                nc.vector.tensor_mul(rhn[:], rz[:, 0:2], ps_ghn[:])
            else:
                hn = ap.tile([128, 2], f32)
                nc.vector.tensor_add(hn[:], ps_ghn[:], bia(8, 10))
                nc.vector.tensor_mul(rhn[:], rz[:, 0:2], hn[:])
            vn = ap.tile([128, 2], f32)
            if zb[4]:
                nc.vector.tensor_add(vn[:], ps_gin[:], rhn[:])
            else:
                t3 = ap.tile([128, 2], f32)
                nc.vector.tensor_add(t3[:], ps_gin[:], bia(10, 12))
                nc.vector.tensor_add(vn[:], t3[:], rhn[:])
            e_n = ap.tile([128, 2], f32)
            nc.scalar.activation(e_n[:], vn[:], AF.Exp, scale=-2.0)
            p_n = ap.tile([128, 2], f32)
            nc.vector.tensor_scalar_add(p_n[:], e_n[:], 1.0)
            r_n = ap.tile([128, 2], f32)
            nc.vector.reciprocal(r_n[:], p_n[:])
            n_t = ap.tile([128, 2], f32)   # n = 2*r - 1
            nc.vector.tensor_scalar(n_t[:], r_n[:], 2.0, 1.0,
                                    ALU.mult, ALU.subtract)
            # h_new = n + z * (h' - n)
            d_t = ap.tile([128, 2], f32)
            nc.vector.tensor_sub(d_t[:], hp[:], n_t[:])
            zd = ap.tile([128, 2], f32)
            nc.vector.tensor_mul(zd[:], rz[:, 2:4], d_t[:])
            h_new = ap.tile([128, 2], f32)
            nc.vector.tensor_add(h_new[:], n_t[:], zd[:])
            # ---- stage 5: logits = log_softmax(w_out @ h_new + b_out)
            # out_w kept fp32 so the matmul reads h_new directly (no cast)
            ps_l = pp.tile([V, 1], f32)
            for kc in range(2):
                nc.tensor.matmul(
                    ps_l[0:V, 0:1],
                    f32sec("out_wf", kc * 29, kc * 29 + 29),
                    h_new[:, kc : kc + 1],
                    start=(kc == 0), stop=(kc == 1),
                )
            # h_new row extraction (bf16 matmul against identity), packed into
            # the output tile; emitted after the logits matmuls.
            nc.tensor.matmul(ps_ap[0:2, 4:132], h_new[0:128, 0:2],
                             f32sec("idf", 0, 128), start=True, stop=True)
            nc.vector.tensor_copy(big[0:2, 64:192], ps_ap[0:2, 4:132])
            # log-softmax in row form: transpose first, sum via ACT accum_out
            l_pad = ap.tile([32, 32], f32)
            nc.vector.memset(l_pad[:], 0.0)
            if zb[5]:
                nc.vector.tensor_copy(l_pad[0:V, 0:1], ps_l[0:V, 0:1])
            else:
                nc.vector.tensor_add(l_pad[0:V, 0:1], ps_l[0:V, 0:1],
                                     bia(3, 4, 0, V))
            l_row = ap.tile([32, 32], f32)
            nc.vector.transpose(l_row[:], l_pad[:])
            e_row = ap.tile([1, 32], f32)
            sx = ap.tile([1, 1], f32)
            nc.scalar.activation(e_row[0:1, 0:V], l_row[0:1, 0:V], AF.Exp,
                                 accum_out=sx[:])
            lse = ap.tile([1, 1], f32)
            nc.scalar.activation(lse[:], sx[:], AF.Ln)
            nc.vector.tensor_scalar(big[0:1, 0:V], l_row[0:1, 0:V], lse[0:1, 0:1],
                                    None, ALU.subtract)
            nc.sync.dma_start(out_d[0:2, 0:192], big[0:2, 0:192])

    _split_multiwaits(nc)
    _hoist_dmas(nc)
    return nc


# ---------------------------------------------------------------- entry point
LAST_EXEC_NS = None


def kernel(input_tensor, hidden, encoder_outputs, cond, is_head,
           emb, w_l2d, b_l2d, w_attn, b_attn, w_comb, b_comb,
           w_ih, w_hh, b_ih, b_hh, w_out, b_out, _trace=False):
    global LAST_EXEC_NS
    from concourse.bass_utils import run_bass_kernel_spmd

    head = bool(np.asarray(is_head).reshape(-1)[0] if np.asarray(is_head).size
                else is_head)
    zb = (
        not np.any(np.asarray(b_attn)),          # attn bias zero
        not np.any(np.asarray(b_comb)),          # comb bias zero
        not (np.any(np.asarray(b_ih)[:512]) or np.any(np.asarray(b_hh)[:512])),
        not np.any(np.asarray(b_hh)[512:]),      # b_hn zero
        not np.any(np.asarray(b_ih)[512:]),      # b_in zero
        not np.any(np.asarray(b_out)),           # out bias zero
    )
    key = ("p", head, zb)
    if key not in _CACHE:
        _CACHE[key] = _build_program(head, zb)
    nc = _CACHE[key]

    mega = _build_mega(input_tensor, hidden, encoder_outputs, cond, emb,
                       w_l2d, b_l2d, w_attn, b_attn, w_comb, b_comb,
                       w_ih, w_hh, b_ih, b_hh, w_out, b_out)
    res = run_bass_kernel_spmd(nc, [{"mega": mega}], core_ids=[0],
                               trace=_trace)
    LAST_EXEC_NS = res.exec_time_ns
    out = np.asarray(res.results[0]["out"], _F32)
    logits = out[0, 0:V].reshape(1, V).copy()
    attn_out = out[0, 32 : 32 + L].reshape(1, L).copy()
    h_out = np.concatenate([out[0, 64:192], out[1, 64:192]]).reshape(1, 1, H)
    return logits, h_out, attn_out


# revision 36
# speedup vs baseline: 1.1819x; 1.0484x over previous
"""Trainium2 Bass kernel for a single AttnDecoderRNN step (batch=1).

Strategy: batch=1 leaves no useful sharding — the whole step runs on one
NeuronCore (core 0). Everything the device needs (weights, hidden state,
encoder outputs, biases) is re-laid-out on the host into a single [128, F]
bf16 "mega" array whose columns are exactly the SBUF images the tensor engine
wants (W^T 128-row chunks for the stationary operand, activations and biases
in column-major [128, c] layout; f32 biases are bit-packed into bf16 column
pairs and bitcast back on device). The device then:

  1. DMAs the mega array in 5 layer-ordered chunks so compute on early layers
     overlaps the DMA of later ones (one DMA ring, minimal issue overhead).
  2. Runs every matvec on the tensor engine in "column" dataflow:
     out[m] = sum_k W^T[k, m] * x[k]  with x as the moving operand, so
     activations stay in [128, c] column layout end-to-end (no transposes).
  3. Uses one activation table set (natural_log_exp_and_others): sigmoid and
     tanh are computed from exp + vector-engine reciprocal; softmax /
     log_softmax use exp/ln with matmul-against-ones partition reductions.
  4. Matmul path runs in bf16 (halves DMA bytes, single-pass PE); gate math,
     biases and all outputs stay fp32.

Embedding row, the two one-hot-selected w_l2d columns, and the is_head branch
are resolved at trace time on the host (they are index selections, not
arithmetic); all floating-point math on activations happens on the device.
"""

import numpy as np
import ml_dtypes

H = 256
L = 20
V = 29

_F32 = np.float32
_BF16 = ml_dtypes.bfloat16

# ---------------------------------------------------------------- mega layout
# Sections, in first-use order. Each entry: (name, n_cols of bf16).
_SECTIONS = [
    ("hid", 2),       # hidden state, column layout [128,2]
    ("biasp", 36),    # fp32 bias block bit-packed as bf16 pairs (18 f32 cols):
                      # f32 cols 0: attn_b | 1-2: comb_b | 3: out_b | 4-7: b_rz
                      # | 8-9: b_hn | 10-11: b_in | 12-13: w_l2dT[256+c0]
                      # | 14-15: w_l2dT[260+c1] | 16-17: b_l2d
    ("l2d_w", 512),   # w_l2d[:, :256].T chunks: block (kc, mc) at kc*256+mc*128
    ("emb", 2),       # emb[idx] in column layout [128,2]
    ("attn_w", 80),   # w_attn.T [512,20] chunks: block kc at kc*20
    ("ones", 130),    # all-ones block: col [*,1] + rows up to [1,128]
    ("enc", 256),     # encoder_outputs [20,256] in rows 0:20
    ("comb_w", 1024), # w_comb.T [512,256] chunks: block (kc,mc) at kc*256+mc*128
    ("ih_w", 1536),   # w_ih.T [256,768] chunks: block (kc,mc) at kc*768+mc*128
    ("hh_w", 1536),   # w_hh.T [256,768] chunks
    ("out_br", 58),   # b_out as an fp32 row at partition 0 (2 bf16 cols per f32)
    ("out_wf", 116),  # w_out.T [256,29] chunks, stored fp32 (2 bf16 cols per f32)
    ("idf", 256),     # fp32 128x128 identity bit-packed (row extraction)
]
_OFF = {}
_cursor = 0
for _name, _n in _SECTIONS:
    _OFF[_name] = _cursor
    _cursor += _n
F_TOTAL = _cursor  # 5072

# DMA group boundaries (column ranges of mega), in layer order.
_GROUPS = [
    (0, _OFF["attn_w"]),                 # hid + biases + l2d + emb   (552)
    (_OFF["attn_w"], _OFF["comb_w"]),    # attn + ones + enc          (366)
    (_OFF["comb_w"], _OFF["ih_w"]),      # comb                       (1024)
    (_OFF["ih_w"], F_TOTAL),             # ih + hh + out + identity   (3386)
]


def _col(v):
    """[128*c] vector -> [128, c] column-major image."""
    v = np.asarray(v, _F32)
    return np.ascontiguousarray(v.reshape(-1, 128).T)


def _colpad(v):
    """[n<=128] vector -> [128, 1], zero padded."""
    out = np.zeros((128, 1), _F32)
    v = np.asarray(v, _F32)
    out[: v.shape[0], 0] = v
    return out


def _chunks(wT):
    """wT [K, O] (K % 128 == 0) -> [128, (K//128)*O] stacked k-chunks."""
    K, O = wT.shape
    return np.concatenate(
        [wT[kc * 128 : (kc + 1) * 128, :] for kc in range(K // 128)], axis=1
    )


def _build_mega(input_tensor, hidden, encoder_outputs, cond, emb, w_l2d,
                b_l2d, w_attn, b_attn, w_comb, b_comb, w_ih, w_hh, b_ih,
                b_hh, w_out, b_out):
    idx = int(np.asarray(input_tensor).reshape(-1)[0])
    c0 = int(np.asarray(cond).reshape(-1)[0])
    c1 = int(np.asarray(cond).reshape(-1)[1])

    mega = np.zeros((128, F_TOTAL), _BF16)

    def put(name, arr):
        off = _OFF[name]
        mega[: arr.shape[0], off : off + arr.shape[1]] = arr.astype(_BF16)

    put("hid", _col(np.asarray(hidden, _F32).reshape(H)))

    w_l2dT = np.ascontiguousarray(np.asarray(w_l2d, _F32).T)  # [264, 256]
    b_ih = np.asarray(b_ih, _F32)
    b_hh = np.asarray(b_hh, _F32)
    biasf = np.zeros((128, 18), _F32)
    biasf[:, 0:1] = _colpad(b_attn)
    biasf[:, 1:3] = _col(b_comb)
    biasf[:, 3:4] = _colpad(b_out)
    biasf[:, 4:8] = _col((b_ih + b_hh)[:512])
    biasf[:, 8:10] = _col(b_hh[512:])
    biasf[:, 10:12] = _col(b_ih[512:])
    biasf[:, 12:14] = _col(w_l2dT[256 + c0])
    biasf[:, 14:16] = _col(w_l2dT[260 + c1])
    biasf[:, 16:18] = _col(b_l2d)
    # bit-preserving pack: each f32 column becomes two bf16 columns
    off = _OFF["biasp"]
    mega[:, off : off + 36] = np.ascontiguousarray(biasf).view(_BF16)

    put("l2d_w", _chunks(w_l2dT[:256, :]))
    put("emb", _col(np.asarray(emb, _F32)[idx]))
    put("attn_w", _chunks(np.ascontiguousarray(np.asarray(w_attn, _F32).T)))
    put("ones", np.ones((128, 130), _F32))
    put("enc", np.asarray(encoder_outputs, _F32))
    put("comb_w", _chunks(np.ascontiguousarray(np.asarray(w_comb, _F32).T)))
    put("ih_w", _chunks(np.ascontiguousarray(np.asarray(w_ih, _F32).T)))
    put("hh_w", _chunks(np.ascontiguousarray(np.asarray(w_hh, _F32).T)))
    off = _OFF["out_br"]
    mega[0:1, off : off + 58] = np.asarray(b_out, _F32).reshape(1, V).view(_BF16)
    off = _OFF["out_wf"]
    mega[:, off : off + 116] = np.ascontiguousarray(
        _chunks(np.ascontiguousarray(np.asarray(w_out, _F32).T))).view(_BF16)
    off = _OFF["idf"]
    mega[:, off : off + 256] = np.eye(128, dtype=_F32).view(_BF16)
    # DMA cost scales with element count, not bytes: ship as f32 (same bits,
    # half the elements -> ~2x stream rate), bitcast back to bf16 on device.
    return np.ascontiguousarray(mega).view(_F32)


# ---------------------------------------------------------------- bass program
_CACHE = {}


def _split_multiwaits(nc):
    """The pinned walrus build rejects >1 sync wait per instruction.  Split
    every multi-wait instruction into a chain of single-wait NOPs on the same
    engine (sequential waits on one engine == logical AND)."""
    import concourse.mybir as mybir

    def fix_block(b):
        insts = b.instructions
        i = 0
        while i < len(insts):
            inst = insts[i]
            si = getattr(inst, "sync_info", None)
            waits = list(si.on_wait) if si is not None and si.on_wait else []
            if len(waits) > 1:
                inst.sync_info = mybir.SyncInfo(
                    on_update=list(si.on_update or []), on_wait=waits[-1:]
                )
                nops = [
                    mybir.InstNoOp(
                        name=f"{inst.name}-sw{k}",
                        engine=inst.engine,
                        bass_nofuse=True,
                        sync_info=mybir.SyncInfo(on_update=[], on_wait=[w]),
                    )
                    for k, w in enumerate(waits[:-1])
                ]
                insts[i:i] = nops
                i += len(nops)
            i += 1
        for sub in getattr(b, "blocks", []) or []:
            fix_block(sub)

    for fn in nc.m.functions:
        for b in fn.blocks:
            fix_block(b)


def _hoist_dmas(nc):
    """Move the mega weight-load DMAs from the Tile body block into the
    preamble ("main") block, ahead of the engine-entry barriers: the SP
    engine then issues them ~0.7us earlier and the transfers overlap the
    rest of the preamble.  Safe because DMA completion is carried by
    semaphore increments that nothing in the preamble resets."""
    fn = nc.m.functions[0]
    pre, body = fn.blocks[0], fn.blocks[1]
    dmas = [i for i in body.instructions
            if i.__class__.__name__ == "InstDMACopy"
            and getattr(i.ins[0], "memref", None) == "mega"][:4]
    if not dmas:
        return
    binsts = body.instructions
    for d in dmas:
        binsts.remove(d)
    pinsts = pre.instructions
    pos = next(
        (k for k, i in enumerate(pinsts)
         if i.__class__.__name__ == "InstEventSemaphore"),
        len(pinsts),
    )
    pinsts[pos:pos] = dmas


def _build_program(is_head, zb):
    import concourse.bass as bass
    import concourse.mybir as mybir
    import concourse.tile as tile

    f32 = mybir.dt.float32
    wdt = mybir.dt.bfloat16
    AF = mybir.ActivationFunctionType
    ALU = mybir.AluOpType

    assert F_TOTAL % 2 == 0
    nc = bass.Bass()
    mega_d = nc.dram_tensor("mega", [128, F_TOTAL // 2], f32,
                            kind="ExternalInput")
    # single packed output: row0 = [logits pad32 | attn pad32 | h[0:128]],
    # row1 = [64 pad | h[128:256]]
    out_d = nc.dram_tensor("out", [2, 192], f32, kind="ExternalOutput")

    with tile.TileContext(nc) as tc:
        with (
            nc.allow_low_precision(reason="bf16 matmul path"),
            tc.tile_pool(name="w", bufs=1) as wp,
            tc.tile_pool(name="a", bufs=1) as ap,
            tc.tile_pool(name="ps", bufs=1, space="PSUM") as pp,
        ):
            gtiles = []
            gviews = []
            for gi_, (lo, hi) in enumerate(_GROUPS):
                t = wp.tile([128, (hi - lo) // 2], f32, tag=f"g{gi_}")
                nc.sync.dma_start(t[:], mega_d[:, lo // 2 : hi // 2])
                gtiles.append(t)
                gviews.append(None)

            def sec(name, r0, r1, p0=0, p1=128):
                """bf16 view of a mega section slice (cols in bf16 units)."""
                off = _OFF[name]
                for g, (lo, hi) in zip(gtiles, _GROUPS):
                    if lo <= off < hi:
                        return g.bitcast(wdt)[p0:p1,
                                              off - lo + r0 : off - lo + r1]
                raise KeyError(name)

            def f32sec(name, r0, r1, p0=0, p1=128):
                """native f32 slice (cols in f32 units)."""
                off = _OFF[name]
                for g, (lo, hi) in zip(gtiles, _GROUPS):
                    if lo <= off < hi:
                        o = (off - lo) // 2
                        return g[p0:p1, o + r0 : o + r1]
                raise KeyError(name)

            def bia(j0, j1, p0=0, p1=128):
                return f32sec("biasp", j0, j1, p0, p1)

            # Dummy early exp so the ACT table set (natural_log_exp) loads
            # while the weight DMAs stream.
            dummy = ap.tile([1, 2], f32)
            nc.vector.memset(dummy[:], 0.0)
            nc.scalar.activation(dummy[:, 0:1], dummy[:, 0:1], AF.Exp)

            big = ap.tile([32, 192], f32)  # packed output rows
            nc.vector.memset(big[:], 0.0)
            # ---- stage 0: h' = w_l2d[:, :256] @ h + (w_l2d col c0 + col c1 + b_l2d)
            hp = ap.tile([128, 2], wdt)  # h' column layout
            hid_c = sec("hid", 0, 2)
            if is_head:
                ebias = ap.tile([128, 2], f32)
                eb1 = ap.tile([128, 2], f32)
                nc.vector.tensor_add(eb1[:], bia(12, 14), bia(14, 16))
                nc.vector.tensor_add(ebias[:], eb1[:], bia(16, 18))
                ps_h = pp.tile([128, 2], f32)
                for mc in range(2):
                    for kc in range(2):
                        nc.tensor.matmul(
                            ps_h[:, mc : mc + 1],
                            sec("l2d_w", kc * 256 + mc * 128,
                                kc * 256 + mc * 128 + 128),
                            hid_c[:, kc : kc + 1],
                            start=(kc == 0), stop=(kc == 1),
                        )
                nc.vector.tensor_add(hp[:], ps_h[:], ebias[:])
            else:
                nc.vector.tensor_copy(hp[:], hid_c)

            # ---- stage 1: attention weights = softmax(w_attn @ [e; h'] + b_attn)
            ps_sm = pp.tile([L, 24], f32)  # 0: logits, 1: sum, 2: bcast, 4:24 row
            e0 = sec("emb", 0, 1)
            e1 = sec("emb", 1, 2)
            rhs_attn = [e0, e1, hp[:, 0:1], hp[:, 1:2]]
            for kc in range(4):
                nc.tensor.matmul(
                    ps_sm[0:L, 0:1],
                    sec("attn_w", kc * 20, kc * 20 + 20),
                    rhs_attn[kc],
                    start=(kc == 0), stop=(kc == 3),
                )
            exp_a = ap.tile([L, 1], wdt)
            nc.scalar.activation(exp_a[:], ps_sm[0:L, 0:1], AF.Exp,
                                 bias=(0.0 if zb[0] else bia(0, 1, 0, L)))
            # attn_applied computed on UNNORMALIZED exp weights; the 1/sum
            # factor is applied to the result via a per-partition scalar.
            ps_ap = pp.tile([128, 132], f32)  # 0-1: ap_raw, 2: 1/sum bcast, 4:132 h row
            for mc in range(2):
                nc.tensor.matmul(
                    ps_ap[:, mc : mc + 1],
                    sec("enc", mc * 128, (mc + 1) * 128, 0, L),
                    exp_a[0:L, 0:1],
                    start=True, stop=True,
                )
            nc.tensor.matmul(ps_sm[0:1, 1:2], exp_a[:], sec("ones", 0, 1, 0, L),
                             start=True, stop=True)
            rsum = ap.tile([1, 1], wdt)
            nc.vector.reciprocal(rsum[:], ps_sm[0:1, 1:2])
            nc.tensor.matmul(ps_ap[0:128, 2:3], sec("ones", 2, 130, 0, 1),
                             rsum[:], start=True, stop=True)
            bc_sb = ap.tile([128, 1], f32)
            nc.vector.tensor_copy(bc_sb[:], ps_ap[0:128, 2:3])
            ap_sb = ap.tile([128, 2], wdt)
            nc.vector.tensor_scalar_mul(ap_sb[:], ps_ap[:, 0:2], bc_sb[:])
            # attention-weights output row (off critical path)
            attn_f = ap.tile([32, 32], f32)
            nc.vector.memset(attn_f[:], 0.0)
            nc.vector.tensor_scalar_mul(attn_f[0:L, 0:1], exp_a[:],
                                        bc_sb[0:L, 0:1])
            nc.vector.transpose(big[0:32, 32:64], attn_f[:])

            # ---- stage 3: u = relu(w_comb @ [e; attn_applied] + b_comb)
            ps_u = pp.tile([128, 2], f32)
            rhs_comb = [e0, e1, ap_sb[:, 0:1], ap_sb[:, 1:2]]
            for mc in range(2):
                for kc in range(4):
                    nc.tensor.matmul(
                        ps_u[:, mc : mc + 1],
                        sec("comb_w", kc * 256 + mc * 128,
                            kc * 256 + mc * 128 + 128),
                        rhs_comb[kc],
                        start=(kc == 0), stop=(kc == 3),
                    )
            u = ap.tile([128, 2], wdt)
            if zb[1]:
                nc.scalar.activation(u[:], ps_u[:, 0:2], AF.Relu)
            else:
                tu = ap.tile([128, 2], f32)
                nc.vector.tensor_add(tu[:], ps_u[:, 0:2], bia(1, 3))
                nc.scalar.activation(u[:], tu[:], AF.Relu)

            # ---- stage 4: GRU cell
            # r/z gates need gi+gh summed — accumulate both weight matmuls
            # into one PSUM group. The n-gate halves stay separate.
            ps_rz = pp.tile([128, 4], f32)
            ps_gin = pp.tile([128, 2], f32)
            ps_ghn = pp.tile([128, 2], f32)
            for mc in range(4):
                for kc in range(2):
                    nc.tensor.matmul(
                        ps_rz[:, mc : mc + 1],
                        sec("ih_w", kc * 768 + mc * 128,
                            kc * 768 + mc * 128 + 128),
                        u[:, kc : kc + 1],
                        start=(kc == 0), stop=False,
                    )
                for kc in range(2):
                    nc.tensor.matmul(
                        ps_rz[:, mc : mc + 1],
                        sec("hh_w", kc * 768 + mc * 128,
                            kc * 768 + mc * 128 + 128),
                        hp[:, kc : kc + 1],
                        start=False, stop=(kc == 1),
                    )
            for mc in range(4, 6):
                for kc in range(2):
                    nc.tensor.matmul(
                        ps_gin[:, mc - 4 : mc - 3],
                        sec("ih_w", kc * 768 + mc * 128,
                            kc * 768 + mc * 128 + 128),
                        u[:, kc : kc + 1],
                        start=(kc == 0), stop=(kc == 1),
                    )
                for kc in range(2):
                    nc.tensor.matmul(
                        ps_ghn[:, mc - 4 : mc - 3],
                        sec("hh_w", kc * 768 + mc * 128,
                            kc * 768 + mc * 128 + 128),
                        hp[:, kc : kc + 1],
                        start=(kc == 0), stop=(kc == 1),
                    )
            # r, z = sigmoid(gi[0:512] + gh[0:512] + b_rz) via exp + reciprocal
            e_rz = ap.tile([128, 4], f32)
            if zb[2]:
                nc.scalar.activation(e_rz[:], ps_rz[:], AF.Exp, scale=-1.0)
            else:
                t2 = ap.tile([128, 4], f32)
                nc.vector.tensor_add(t2[:], ps_rz[:], bia(4, 8))
                nc.scalar.activation(e_rz[:], t2[:], AF.Exp, scale=-1.0)
            p_rz = ap.tile([128, 4], f32)
            nc.vector.tensor_scalar_add(p_rz[:], e_rz[:], 1.0)
            rz = ap.tile([128, 4], f32)
            nc.vector.reciprocal(rz[:], p_rz[:])
            # n = tanh(gi[512:768] + b_in + r * (gh[512:768] + b_hn))
            rhn = ap.tile([128, 2], f32)
            if zb[3]:
                nc.vector.tensor_mul(rhn[:], rz[:, 0:2], ps_ghn[:])
            else:
                hn = ap.tile([128, 2], f32)
                nc.vector.tensor_add(hn[:], ps_ghn[:], bia(8, 10))
                nc.vector.tensor_mul(rhn[:], rz[:, 0:2], hn[:])
            vn = ap.tile([128, 2], f32)
            if zb[4]:
                nc.vector.tensor_add(vn[:], ps_gin[:], rhn[:])
            else:
                t3 = ap.tile([128, 2], f32)
                nc.vector.tensor_add(t3[:], ps_gin[:], bia(10, 12))
                nc.vector.tensor_add(vn[:], t3[:], rhn[:])
            e_n = ap.tile([128, 2], f32)
            nc.scalar.activation(e_n[:], vn[:], AF.Exp, scale=-2.0)
            p_n = ap.tile([128, 2], f32)
            nc.vector.tensor_scalar_add(p_n[:], e_n[:], 1.0)
            r_n = ap.tile([128, 2], f32)
            nc.vector.reciprocal(r_n[:], p_n[:])
            n_t = ap.tile([128, 2], f32)   # n = 2*r - 1
            nc.vector.tensor_scalar(n_t[:], r_n[:], 2.0, 1.0,
                                    ALU.mult, ALU.subtract)
            # h_new = n + z * (h' - n)
            d_t = ap.tile([128, 2], f32)
            nc.vector.tensor_sub(d_t[:], hp[:], n_t[:])
            zd = ap.tile([128, 2], f32)
            nc.vector.tensor_mul(zd[:], rz[:, 2:4], d_t[:])
            h_new = ap.tile([128, 2], f32)
            nc.vector.tensor_add(h_new[:], n_t[:], zd[:])
            # ---- stage 5: logits = log_softmax(w_out @ h_new + b_out)
            # Row-form: h-column chunks as the stationary operand, fp32 w_out
            # streaming -> logits land as a [1,V] psum ROW (no transpose).
            ps_l = pp.tile([1, 32], f32)
            for kc in range(2):
                nc.tensor.matmul(
                    ps_l[0:1, 0:V],
                    h_new[:, kc : kc + 1],
                    f32sec("out_wf", kc * 29, kc * 29 + 29),
                    start=(kc == 0), stop=(zb[5] and kc == 1),
                )
            if not zb[5]:
                nc.tensor.matmul(ps_l[0:1, 0:V], f32sec("ones", 0, 1, 0, 1),
                                 f32sec("out_br", 0, V, 0, 1),
                                 start=False, stop=True)
            # h_new row extraction (fp32 identity matmuls), packed into big
            nc.tensor.matmul(ps_ap[0:2, 4:132], h_new[0:128, 0:2],
                             f32sec("idf", 0, 128), start=True, stop=True)
            nc.vector.tensor_copy(big[0:2, 64:192], ps_ap[0:2, 4:132])
            e_row = ap.tile([1, 32], f32)
            sx = ap.tile([1, 1], f32)
            nc.scalar.activation(e_row[0:1, 0:V], ps_l[0:1, 0:V], AF.Exp,
                                 accum_out=sx[:])
            lse = ap.tile([1, 1], f32)
            nc.scalar.activation(lse[:], sx[:], AF.Ln)
            nc.vector.tensor_scalar(big[0:1, 0:V], ps_l[0:1, 0:V],
                                    lse[0:1, 0:1], None, ALU.subtract)
            nc.sync.dma_start(out_d[0:2, 0:192], big[0:2, 0:192])

    _split_multiwaits(nc)
    _hoist_dmas(nc)
    return nc


# ---------------------------------------------------------------- entry point
LAST_EXEC_NS = None


def kernel(input_tensor, hidden, encoder_outputs, cond, is_head,
           emb, w_l2d, b_l2d, w_attn, b_attn, w_comb, b_comb,
           w_ih, w_hh, b_ih, b_hh, w_out, b_out, _trace=False):
    global LAST_EXEC_NS
    from concourse.bass_utils import run_bass_kernel_spmd

    head = bool(np.asarray(is_head).reshape(-1)[0] if np.asarray(is_head).size
                else is_head)
    zb = (
        not np.any(np.asarray(b_attn)),          # attn bias zero
        not np.any(np.asarray(b_comb)),          # comb bias zero
        not (np.any(np.asarray(b_ih)[:512]) or np.any(np.asarray(b_hh)[:512])),
        not np.any(np.asarray(b_hh)[512:]),      # b_hn zero
        not np.any(np.asarray(b_ih)[512:]),      # b_in zero
        not np.any(np.asarray(b_out)),           # out bias zero
    )
    key = ("p", head, zb)
    if key not in _CACHE:
        _CACHE[key] = _build_program(head, zb)
    nc = _CACHE[key]

    mega = _build_mega(input_tensor, hidden, encoder_outputs, cond, emb,
                       w_l2d, b_l2d, w_attn, b_attn, w_comb, b_comb,
                       w_ih, w_hh, b_ih, b_hh, w_out, b_out)
    res = run_bass_kernel_spmd(nc, [{"mega": mega}], core_ids=[0],
                               trace=_trace)
    LAST_EXEC_NS = res.exec_time_ns
    out = np.asarray(res.results[0]["out"], _F32)
    logits = out[0, 0:V].reshape(1, V).copy()
    attn_out = out[0, 32 : 32 + L].reshape(1, L).copy()
    h_out = np.concatenate([out[0, 64:192], out[1, 64:192]]).reshape(1, 1, H)
    return logits, h_out, attn_out
